# revision 8
# baseline (speedup 1.0000x reference)
"""Trainium2 Bass kernel for nn_DGCRM_88227218194820.

The reference module's dynamic-adjacency branch (gconv_hyper / nodevec /
adp) is dead code w.r.t. the returned hidden state: due to the faithful
source bug, gconv_rnn(inp, i) == concat([inp, a*inp, a*inp], -1) @ rnn_W[i]
+ rnn_b[i] uses no adjacency, and the normalized adjacencies are deleted.
The output therefore reduces to a per-row GRU gate:

    combined = concat(x, h)                      # [.., 66]
    z  = sigmoid(combined @ Wz + bz)
    r  = sigmoid(combined @ Wr + br)
    hc = tanh(concat(x, r*h) @ Wc + bc)
    out = z*h + (1-z)*hc

with Wg folded from rnn_W: Wg = W[:66] + a*(W[66:132] + W[132:198]),
summed over the two gconv_rnn calls per gate.

Layout (per core, data-parallel over batch: 2 of 16 batches per core,
R = 2048 rows): transposed (channels on partitions), group-stacked --
rows 0:1024 (group A) on partitions 0:64, rows 1024:2048 (group B) on
partitions 64:128.

Matmuls run in fp8e4 + MatmulPerfMode.DoubleRow: K is spread over two
k-slices of 128 partitions (virtual 256-row array), so the x channels +
folded bias ride in a second k-slice of the SAME matmul instead of the
baseline's separate K=6 passes -- 6 matmuls total instead of 12.  The
rhs k-pairs come from one 3D fp8 tile XHR[128, 3, 1024], dim1 =
(rh, x, h): z/r read [:, 1:3] = (x, h), the candidate reads [:, 0:2] =
(rh, x); the x slice is shared and rh is DVE-written (r (x) h, fp8) in
place.  fp8's ~2% quantization error on the dominant x-terms is killed
by error-compensation channel triples (qW,qx),(dW,qx),(qW,dx); the
h-side fp8 error survives; measured end-to-end rel err ~1.1e-2 (< 2e-2).

h is additionally shipped as bf16 for the final blend (z*h needs
full-precision h).

Perf notes (from NTFF traces):
 - input DMA transfer rate is descriptor-bound (~10ns/row): ship few
   DMAs with wide rows (768B / 2048B / 2048B), all on the Sync HWDGE;
   a 4th concurrent DMA on the GpSimd path paced everything down.
 - PSUM dependency tracking is tile-granular: per-chunk psum tiles
   (pr0/pr1, pc0/pc1) so chunk-0 activations don't wait on chunk-1
   matmuls.  SBUF tracking is range-based, so SBUF tiles are shared.
 - ACT FIFO order r0, r1, z, t0, t1a, t1b: the off-critical-path z
   sigmoid (1 op, 1024 wide) fills the gap while DVE computes rh and
   the PE runs the candidate matmuls; tanh tail at 256 granularity
   feeds the blend/output pipeline early.
"""

import ml_dtypes
import numpy as np

import concourse.tile as tile
from concourse import bacc, mybir
from concourse.bass_utils import run_bass_kernel_spmd

N_CORES = 8
B, N, IN_DIM, HID = 16, 1024, 2, 64
GC_ALPHA = 0.05
CIN = HID + IN_DIM          # 66
R = (B // N_CORES) * N      # 2048 rows per core
G = R // 2                  # 1024 rows per group (A/B)
BLK = 512
N_WARMUP_MM = 2

F32 = mybir.dt.float32
BF16 = mybir.dt.bfloat16
FP8 = mybir.dt.float8e4
AF = mybir.ActivationFunctionType
DR = mybir.MatmulPerfMode.DoubleRow
BF16_NP = ml_dtypes.bfloat16
FP8_NP = ml_dtypes.float8_e4m3fn

_program_cache = {}


def build_program():
    nc = bacc.Bacc()
    w3d = nc.dram_tensor("w3d", [128, 3, 2, 128], FP8, kind="ExternalInput")
    xsl = nc.dram_tensor("xsl", [128, G], FP8, kind="ExternalInput")
    hf8 = nc.dram_tensor("hf8", [128, G], FP8, kind="ExternalInput")
    htb = nc.dram_tensor("htb", [128, G], BF16, kind="ExternalInput")
    ot = nc.dram_tensor("ot", [128, G], BF16, kind="ExternalOutput")

    with tile.TileContext(nc) as tc:
        with (
            tc.tile_pool(name="sb", bufs=1) as sb,
            tc.tile_pool(name="ps", bufs=1, space="PSUM") as ps,
        ):
            W3 = sb.tile([128, 3, 2, 128], FP8, tag="W3")
            # dim1: 0 = rh (DVE-written), 1 = x slice, 2 = h slice
            XHR = sb.tile([128, 3, G], FP8, tag="XHR")
            HT = sb.tile([128, G], BF16, tag="HT")
            RT = sb.tile([128, G], FP8, tag="RT")
            ZT = sb.tile([128, G], BF16, tag="ZT")
            HC = sb.tile([128, G], BF16, tag="HC")
            DD = sb.tile([128, G], BF16, tag="DD")
            ZD = sb.tile([128, G], BF16, tag="ZD")
            OT = sb.tile([128, G], BF16, tag="OT")
            WARM = sb.tile([128, 2, BLK], FP8, tag="WARM")
            dummy = sb.tile([1, 1], F32, tag="dummy")

            pr0 = ps.tile([128, BLK], F32, tag="pr0")
            pr1 = ps.tile([128, BLK], F32, tag="pr1")
            pz = ps.tile([128, G], F32, tag="pz")
            pc0 = ps.tile([128, BLK], F32, tag="pc0")
            pc1 = ps.tile([128, BLK], F32, tag="pc1")
            pw = ps.tile([128, BLK], F32, tag="pw")

            # Fire the ACT table load (sigmoid_and_others covers tanh)
            # immediately so it overlaps the input DMAs.
            nc.vector.memset(dummy, 0.0)
            nc.scalar.activation(
                out=dummy, in_=dummy, func=AF.Sigmoid, bias=dummy[0:1, 0:1]
            )

            # Input DMAs in need-order on the Sync HWDGE; h-fp8 split in
            # halves so the chunk-0 matmuls start before all of h lands.
            nc.sync.dma_start(out=W3[:, :, :, :], in_=w3d[:, :, :, :])
            nc.sync.dma_start(out=XHR[:, 1, :], in_=xsl[:, :])
            nc.sync.dma_start(out=XHR[:, 2, 0:BLK], in_=hf8[:, 0:BLK])
            nc.sync.dma_start(out=XHR[:, 2, BLK:G], in_=hf8[:, BLK:G])
            nc.sync.dma_start(out=HT, in_=htb[:, :])

            # PE warm-up on zeroed fp8 while DMAs fly; scribbles into pw
            # which nothing reads.
            nc.vector.memset(WARM, 0.0)
            for _ in range(N_WARMUP_MM):
                nc.tensor.matmul(
                    pw[:, :], WARM[:, :, 0:128], WARM[:, :, :],
                    start=True, stop=True, perf_mode=DR, skip_group_check=True,
                )

            c0 = slice(0, BLK)
            c1 = slice(BLK, G)
            half = BLK // 2
            c1a = slice(BLK, BLK + half)
            c1b = slice(BLK + half, G)

            def mm(psum_t, n, g, kpair, cs):
                nc.tensor.matmul(
                    psum_t[:, 0:n], W3[:, g, :, :], XHR[:, kpair, cs],
                    start=True, stop=True, perf_mode=DR, skip_group_check=True,
                )

            ZR = slice(1, 3)   # (x, h) k-pair for z/r
            CC = slice(0, 2)   # (rh, x) k-pair for candidate

            # gate indices: 0=z, 1=r, 2=c
            mm(pr0, BLK, 1, ZR, c0)
            mm(pr1, BLK, 1, ZR, c1)
            for cs in (c0, c1):
                nc.tensor.matmul(
                    pz[:, cs], W3[:, 0, :, :], XHR[:, ZR, cs],
                    start=True, stop=True, perf_mode=DR, skip_group_check=True,
                )

            nc.scalar.activation(out=RT[:, c0], in_=pr0[:, :], func=AF.Sigmoid)
            nc.scalar.activation(out=RT[:, c1], in_=pr1[:, :], func=AF.Sigmoid)

            nc.vector.tensor_mul(XHR[:, 0, c0], RT[:, c0], XHR[:, 2, c0])
            nc.vector.tensor_mul(XHR[:, 0, c1], RT[:, c1], XHR[:, 2, c1])

            mm(pc0, BLK, 2, CC, c0)
            mm(pc1, BLK, 2, CC, c1)

            # ACT FIFO: z fills the c-matmul wait gap; tanh tail at 256.
            nc.scalar.activation(out=ZT[:, :], in_=pz[:, :], func=AF.Sigmoid)
            nc.scalar.activation(out=HC[:, c0], in_=pc0[:, :], func=AF.Tanh)
            nc.scalar.activation(out=HC[:, c1a], in_=pc1[:, 0:half], func=AF.Tanh)
            nc.scalar.activation(out=HC[:, c1b], in_=pc1[:, half:BLK], func=AF.Tanh)

            # blend: out = hc + z*(h - hc); chunk 0 wide, tail at 256
            nc.vector.tensor_sub(DD[:, c0], HT[:, c0], HC[:, c0])
            nc.vector.tensor_mul(ZD[:, c0], ZT[:, c0], DD[:, c0])
            nc.vector.tensor_add(OT[:, c0], HC[:, c0], ZD[:, c0])
            nc.sync.dma_start(out=ot[:, c0], in_=OT[:, c0])

            for cs in (c1a, c1b):
                nc.vector.tensor_sub(DD[:, cs], HT[:, cs], HC[:, cs])
                nc.vector.tensor_mul(ZD[:, cs], ZT[:, cs], DD[:, cs])
                nc.vector.tensor_add(OT[:, cs], HC[:, cs], ZD[:, cs])
            nc.sync.dma_start(out=ot[:, c1], in_=OT[:, c1])

    nc.compile()
    return nc


def get_program():
    if "nc" not in _program_cache:
        _program_cache["nc"] = build_program()
    return _program_cache["nc"]


def fold_params(rnn_W, rnn_b):
    """Fold the gconv_rnn bug + gate sums into per-gate [66,64] weights."""
    Wf = rnn_W[:, :CIN, :] + GC_ALPHA * (
        rnn_W[:, CIN : 2 * CIN, :] + rnn_W[:, 2 * CIN : 3 * CIN, :]
    )  # [6, 66, 64]
    Wg = np.stack([Wf[0] + Wf[1], Wf[2] + Wf[3], Wf[4] + Wf[5]])  # [3,66,64]
    bg = np.stack(
        [rnn_b[0] + rnn_b[1], rnn_b[2] + rnn_b[3], rnn_b[4] + rnn_b[5]]
    )  # [3, 64]
    return Wg, bg


def _q8(a):
    return a.astype(FP8_NP)


def pack_weights(rnn_W, rnn_b):
    """[128, 3, 2, 128] fp8 weight block per the kernel layout."""
    Wg, bg = fold_params(rnn_W, rnn_b)
    W_x = Wg[:, :IN_DIM, :]   # [3, 2, 64]
    W_h = Wg[:, IN_DIM:, :]   # [3, 64, 64]

    w3 = np.zeros((128, 3, 2, 128), FP8_NP)
    for g in range(3):
        # blockdiag slice: group A rows 0:64 -> outs 0:64, B rows 64:128
        # -> outs 64:128.  z/r rhs pair = (x, h) -> blockdiag in slice 1;
        # c rhs pair = (rh, x) -> blockdiag in slice 0.
        sb_ = 1 if g < 2 else 0
        sx = 1 - sb_
        qwh = _q8(W_h[g])
        w3[0:64, g, sb_, 0:64] = qwh
        w3[64:128, g, sb_, 64:128] = qwh
        # x-slice channels (per group base p0, out span os):
        #   p0+0/1: qx*qW   p0+2/3: qx*dW   p0+4/5: dx*qW   p0+6: 1*q(b)
        qw = _q8(W_x[g])                       # [2, 64]
        dw = _q8(W_x[g] - qw.astype(np.float32))
        for p0, os_ in ((0, slice(0, 64)), (64, slice(64, 128))):
            w3[p0 + 0 : p0 + 2, g, sx, os_] = qw
            w3[p0 + 2 : p0 + 4, g, sx, os_] = dw
            w3[p0 + 4 : p0 + 6, g, sx, os_] = qw
            w3[p0 + 6, g, sx, os_] = _q8(bg[g])
    return w3


def make_in_maps(x, h, rnn_W, rnn_b):
    w3 = pack_weights(rnn_W, rnn_b)
    hf = h.reshape(N_CORES, R, HID)
    xf = x.reshape(N_CORES, R, IN_DIM)
    in_maps = []
    for c in range(N_CORES):
        ht_host = np.ascontiguousarray(
            np.concatenate([hf[c, :G].T, hf[c, G:].T], axis=0)
        )  # [128, G] f32, group-stacked
        xsl_host = np.zeros((128, G), FP8_NP)
        for p0, rows in ((0, xf[c, :G]), (64, xf[c, G:])):
            qx = _q8(rows.T)                               # [2, G]
            dx = _q8(rows.T - qx.astype(np.float32))
            xsl_host[p0 + 0 : p0 + 2] = qx
            xsl_host[p0 + 2 : p0 + 4] = qx
            xsl_host[p0 + 4 : p0 + 6] = dx
            xsl_host[p0 + 6] = 1.0
        in_maps.append(
            dict(
                w3d=w3,
                xsl=xsl_host,
                hf8=_q8(ht_host),
                htb=ht_host.astype(BF16_NP),
            )
        )
    return in_maps


def gather_output(results):
    outs = []
    for c in range(N_CORES):
        o = np.asarray(results[c]["ot"]).astype(np.float32)  # [128, G]
        outs.append(np.concatenate([o[:64].T, o[64:].T], axis=0))  # [R, HID]
    return (
        np.concatenate(outs, axis=0).reshape(B, N, HID).astype(np.float32)
    )


def run(inputs, trace=False, **kw):
    x = np.ascontiguousarray(np.asarray(inputs["x"], dtype=np.float32))
    h = np.ascontiguousarray(
        np.asarray(inputs["hidden_state"], dtype=np.float32)
    )
    rnn_W = np.asarray(inputs["rnn_W"], dtype=np.float32)
    rnn_b = np.asarray(inputs["rnn_b"], dtype=np.float32)

    in_maps = make_in_maps(x, h, rnn_W, rnn_b)
    nc = get_program()
    res = run_bass_kernel_spmd(
        nc, in_maps, core_ids=list(range(N_CORES)), trace=trace, **kw
    )
    return gather_output(res.results), res


def kernel(**inputs) -> np.ndarray:
    out, _ = run(inputs)
    return out
